# revision 4
# baseline (speedup 1.0000x reference)
"""Trainium2 Bass kernel: 8-layer ternary (BitNet-1.58) dense transformer.

Model (per reference):
    h = embed[input_ids]                                  # (B=2, S=1024, H=2048)
    8x: y = h @ ternary(W_l)^T + b_l ; h = LN(y + h)*g+b  # H=2048
    h = LN(h)*final_g + final_b
    logits = h @ ternary(head_W)^T                        # (B, S, V=32000)

Sharding over 8 NeuronCores (fully local, no collectives):
  - Layers: data-parallel over the 2048 tokens (256 tokens/core). Each core
    streams the full (bf16 ternary) layer weights.
  - Head: ALSO data-parallel over tokens: each core computes its own 256
    tokens x the full 32000-entry vocab. The ternary head weights are sent
    as exact {-1,0,+1} fp8(e4m3) and streamed chunk-by-chunk, overlapped
    with compute. No AllGather.

Head matmul runs mixed-precision: k-tiles 0..5 via fp8 DoubleRow (2 k-tiles
per instruction, activations rounded to e4m3), k-tiles 6..15 as bf16
activations x fp8 weights at full precision. The e4m3 rounding of 6/16 of
the contraction costs ~1.6e-2 relative error on the logits (vs the 2e-2
budget) and saves ~16% of the dominant matmul stream time.
"""

import os
import sys

import numpy as np

try:
    import concourse.bass as bass
except ImportError:  # grading container should have it on sys.path already
    sys.path.insert(0, "/opt/trn_rl_repo")
    import concourse.bass as bass

import ml_dtypes
import concourse.mybir as mybir
import concourse.tile as tile
from concourse import bacc
from concourse.bass_utils import run_bass_kernel_spmd
from contextlib import ExitStack

F32 = mybir.dt.float32
BF16 = mybir.dt.bfloat16
FP8E4 = mybir.dt.float8e4
AX = mybir.AxisListType
OP = mybir.AluOpType
AF = mybir.ActivationFunctionType
DR = mybir.MatmulPerfMode.DoubleRow
EPS = 1e-5

# Full-size problem config (B=2, S=1024 -> 2048 tokens, 256/core).
# Head: vocab padded 32000 -> 63*512; k-tiles 0..DRKT-1 run as fp8 DoubleRow.
CFG_FULL = dict(L=8, H=2048, NC=8, TT=2, V=32000, QV=512, NQ=63, CH=512, DRKT=6)


def build_nc(cfg, scales, head_scale, triv_ln, fp8_w, use_dr):
    L, H, NC, TT = cfg["L"], cfg["H"], cfg["NC"], cfg["TT"]
    V, QV, NQ, CH, DRKT = cfg["V"], cfg["QV"], cfg["NQ"], cfg["CH"], cfg["DRKT"]
    KT = H // 128
    KQ = KT // 4  # k-tiles per layer-weight quarter
    NCH = H // CH
    DRP = DRKT // 2
    if not use_dr:
        DRKT = DRP = 0
    assert H % CH == 0 and NQ * QV >= V
    WDT = FP8E4 if fp8_w else BF16

    nc = bacc.Bacc("TRN2", target_bir_lowering=False, debug=False, num_devices=NC)
    h0 = nc.declare_dram_parameter("h0", [TT, 128, H], F32, isOutput=False)
    h0T = nc.declare_dram_parameter("h0T", [TT, 128, H], BF16, isOutput=False)
    w_ = nc.declare_dram_parameter("w", [L, KT, 128, H], WDT, isOutput=False)
    if not triv_ln:
        lng = nc.declare_dram_parameter("lng", [L, H], BF16, isOutput=False)
        lnb = nc.declare_dram_parameter("lnb", [L, H], BF16, isOutput=False)
        lbias = nc.declare_dram_parameter("lbias", [L, H], BF16, isOutput=False)
        fing = nc.declare_dram_parameter("fing", [H], BF16, isOutput=False)
        finb = nc.declare_dram_parameter("finb", [H], BF16, isOutput=False)
    hw_ = nc.declare_dram_parameter("hw", [NQ, 128, KT, QV], WDT, isOutput=False)
    ident_d = nc.declare_dram_parameter("ident", [128, 128], F32, isOutput=False)
    eps_d = nc.declare_dram_parameter("eps", [128, 1], F32, isOutput=False)
    out = nc.declare_dram_parameter("out", [TT * 128, V], F32, isOutput=True)

    with tile.TileContext(nc) as tc:
        with ExitStack() as ctx0:
            consts = ctx0.enter_context(tc.tile_pool(name="consts", bufs=1))
            state = ctx0.enter_context(tc.tile_pool(name="state", bufs=4))
            hTp = ctx0.enter_context(tc.tile_pool(name="hT", bufs=2))
            hT8p = ctx0.enter_context(tc.tile_pool(name="hT8", bufs=2))
            wqp = ctx0.enter_context(tc.tile_pool(name="wq", bufs=3))
            outp = ctx0.enter_context(tc.tile_pool(name="outstg", bufs=4))
            smp = ctx0.enter_context(tc.tile_pool(name="small", bufs=16))

            ident = consts.tile([128, 128], F32)
            nc.sync.dma_start(ident[:], ident_d[:])
            eps_t = consts.tile([128, 1], F32)
            nc.sync.dma_start(eps_t[:], eps_d[:])

            h_cur = []
            hT_cur = []
            for t in range(TT):
                # pre-transposed & pre-scaled (layer-0 ternary scale) lhsT
                hTt = hTp.tile([128, H], BF16, tag="hT", name=f"hT_p{t}")
                nc.sync.dma_start(hTt[:], h0T[t])
                hT_cur.append(hTt)
                st = state.tile([128, H], F32, name=f"hinit{t}", tag="state")
                nc.sync.dma_start(st[:], h0[t])
                h_cur.append(st)
            hT8_cur = [None] * TT

            with ExitStack() as ctxA:
                zpool = ctxA.enter_context(tc.tile_pool(name="z", bufs=2))
                wp = ctxA.enter_context(tc.tile_pool(name="w", bufs=6))
                sqp = ctxA.enter_context(tc.tile_pool(name="sq", bufs=2))
                gbp = None
                if not triv_ln:
                    gbp = ctxA.enter_context(tc.tile_pool(name="gb", bufs=2))
                psT = ctxA.enter_context(
                    tc.tile_pool(name="psT", bufs=2, space="PSUM")
                )
                psY = ctxA.enter_context(
                    tc.tile_pool(name="psY", bufs=NCH, space="PSUM")
                )

                def transpose_cast(src_f32, scale_imm, name, final=False):
                    """h [128tok, H] f32 -> hT [128feat-in-blk, (kt,128tok)]
                    bf16 * s, split into 2 halves so the scalar copy of half
                    0 overlaps the PE transposes of half 1. For the final
                    (head) activations also emit an UNSCALED e4m3 copy of
                    k-tiles 0..DRKT-1 for the DoubleRow part."""
                    dst = hTp.tile([128, H], BF16, tag="hT", name=f"hT{name}")
                    dst8 = None
                    if final and DRKT:
                        dst8 = hT8p.tile(
                            [128, DRKT, 128], FP8E4, tag="hT8", name=f"hT8{name}"
                        )
                    for hf in range(2):
                        pT = psT.tile(
                            [128, H // 2], F32, tag="psT", name=f"pT{name}_{hf}"
                        )
                        for k in range(KT // 2):
                            kt = hf * (KT // 2) + k
                            nc.tensor.transpose(
                                pT[:, k * 128 : (k + 1) * 128],
                                src_f32[:, kt * 128 : (kt + 1) * 128],
                                ident[:],
                            )
                        lo = hf * (H // 2)
                        if dst8 is not None and hf == 0:
                            nc.scalar.activation(
                                dst8[:].rearrange("p a b -> p (a b)"),
                                pT[:, : DRKT * 128],
                                AF.Copy,
                            )
                            nc.scalar.activation(
                                dst[:, DRKT * 128 : H // 2],
                                pT[:, DRKT * 128 :],
                                AF.Copy,
                                scale=float(scale_imm),
                            )
                        else:
                            nc.scalar.activation(
                                dst[:, lo : lo + H // 2],
                                pT[:],
                                AF.Copy,
                                scale=float(scale_imm),
                            )
                    return dst, dst8

                def ln_finish(z, S_ap, SS_ap, g_t, b_t, name):
                    S = smp.tile([128, 1], F32, tag="s0", name=f"S{name}")
                    SS = smp.tile([128, 1], F32, tag="s1", name=f"SS{name}")
                    nc.vector.tensor_reduce(S[:], S_ap, axis=AX.X, op=OP.add)
                    nc.vector.tensor_reduce(SS[:], SS_ap, axis=AX.X, op=OP.add)
                    negmean = smp.tile([128, 1], F32, tag="s2", name=f"nm{name}")
                    nc.vector.tensor_scalar_mul(negmean[:], S[:], -1.0 / H)
                    msq = smp.tile([128, 1], F32, tag="s3", name=f"msq{name}")
                    nc.vector.tensor_scalar_mul(msq[:], SS[:], 1.0 / H)
                    var = smp.tile([128, 1], F32, tag="s4", name=f"var{name}")
                    nc.vector.tensor_tensor(var[:], negmean[:], negmean[:], OP.mult)
                    nc.vector.tensor_tensor(var[:], msq[:], var[:], OP.subtract)
                    std = smp.tile([128, 1], F32, tag="s5", name=f"std{name}")
                    nc.scalar.activation(std[:], var[:], AF.Sqrt, bias=eps_t[:])
                    rstd = smp.tile([128, 1], F32, tag="s6", name=f"rstd{name}")
                    nc.vector.reciprocal(rstd[:], std[:])
                    hn = state.tile([128, H], F32, tag="state", name=f"h{name}")
                    nc.vector.tensor_scalar(
                        hn[:], z[:], negmean[:], rstd[:], OP.add, OP.mult
                    )
                    if g_t is not None:
                        nc.vector.tensor_tensor(hn[:], hn[:], g_t[:], OP.mult)
                        nc.vector.tensor_tensor(hn[:], hn[:], b_t[:], OP.add)
                    return hn

                def stats_of(src, name):
                    """LN stats for an existing tensor (final LN input)."""
                    sums = smp.tile(
                        [128, 2 * NCH], F32, tag="sums", name=f"sm{name}"
                    )
                    nc.vector.tensor_reduce(
                        sums[:, 0:1], src[:], axis=AX.X, op=OP.add
                    )
                    for i in range(NCH):
                        sq = sqp.tile(
                            [128, CH], BF16, tag="sq", name=f"sq{name}_{i}"
                        )
                        nc.scalar.activation(
                            sq[:],
                            src[:, i * CH : (i + 1) * CH],
                            AF.Square,
                            accum_out=sums[:, NCH + i : NCH + i + 1],
                        )
                    return sums

                fg = fb = None
                if not triv_ln:
                    fg = gbp.tile([128, H], BF16, tag="g", name="gfin")
                    nc.sync.dma_start(fg[:], fing[None, :].to_broadcast((128, H)))
                    fb = gbp.tile([128, H], BF16, tag="b", name="bfin")
                    nc.sync.dma_start(fb[:], finb[None, :].to_broadcast((128, H)))

                for l in range(L):
                    w_q = []
                    for hf in range(4):
                        wt = wp.tile([128, KQ, H], WDT, tag="w", name=f"w{l}_{hf}")
                        nc.sync.dma_start(
                            wt[:],
                            w_[l, hf * KQ : (hf + 1) * KQ].rearrange(
                                "k p o -> p k o"
                            ),
                        )
                        w_q.append(wt)
                    g_t = b_t = bias_t = None
                    if not triv_ln:
                        g_t = gbp.tile([128, H], BF16, tag="g", name=f"g{l}")
                        nc.sync.dma_start(
                            g_t[:], lng[l][None, :].to_broadcast((128, H))
                        )
                        b_t = gbp.tile([128, H], BF16, tag="b", name=f"b{l}")
                        nc.sync.dma_start(
                            b_t[:], lnb[l][None, :].to_broadcast((128, H))
                        )
                        bias_t = gbp.tile(
                            [128, H], BF16, tag="bias", name=f"bias{l}"
                        )
                        nc.sync.dma_start(
                            bias_t[:], lbias[l][None, :].to_broadcast((128, H))
                        )

                    last = l == L - 1
                    next_scale = scales[l + 1] if not last else None
                    for t in range(TT):
                        hTt = hT_cur[t]
                        ps = [
                            psY.tile([128, CH], F32, tag="psY", name=f"ps{l}_{t}_{i}")
                            for i in range(NCH)
                        ]
                        for half in range(2):
                            for kt in range(KT):
                                wt = w_q[kt // KQ]
                                for i in (2 * half, 2 * half + 1):
                                    nc.tensor.matmul(
                                        ps[i][:],
                                        lhsT=hTt[:, kt * 128 : (kt + 1) * 128],
                                        rhs=wt[:, kt % KQ, i * CH : (i + 1) * CH],
                                        start=(kt == 0),
                                        stop=(kt == KT - 1),
                                    )
                        z = zpool.tile([128, H], F32, tag="z", name=f"z{l}_{t}")
                        sums = smp.tile(
                            [128, 2 * NCH], F32, tag="sums", name=f"sm{l}_{t}"
                        )
                        resid = h_cur[t]
                        if not triv_ln:
                            hb = zpool.tile(
                                [128, H], F32, tag="hb", name=f"hb{l}_{t}"
                            )
                            nc.vector.tensor_tensor(
                                hb[:], h_cur[t][:], bias_t[:], OP.add
                            )
                            resid = hb
                        for i in range(NCH):
                            # fused: z = ps + resid AND chunk-sum accumulate
                            nc.vector.tensor_tensor_reduce(
                                z[:, i * CH : (i + 1) * CH],
                                ps[i][:],
                                resid[:, i * CH : (i + 1) * CH],
                                1.0,
                                0.0,
                                OP.add,
                                OP.add,
                                accum_out=sums[:, i : i + 1],
                            )
                        for i in range(NCH):
                            sq = sqp.tile(
                                [128, CH], BF16, tag="sq", name=f"sq{l}_{t}_{i}"
                            )
                            nc.scalar.activation(
                                sq[:],
                                z[:, i * CH : (i + 1) * CH],
                                AF.Square,
                                accum_out=sums[:, NCH + i : NCH + i + 1],
                            )
                        hn = ln_finish(
                            z, sums[:, 0:NCH], sums[:, NCH : 2 * NCH],
                            g_t, b_t, f"{l}_{t}",
                        )
                        h_cur[t] = hn
                        if not last:
                            hT_cur[t], _ = transpose_cast(
                                hn, next_scale, f"{l}_{t}"
                            )
                        else:
                            # final LN chain emitted immediately (vector/
                            # scalar work overlaps the other tile's matmuls)
                            fsums = stats_of(hn, f"f{t}")
                            hfin = ln_finish(
                                hn, fsums[:, 0:1], fsums[:, NCH : 2 * NCH],
                                fg, fb, f"fin{t}",
                            )
                            h_cur[t] = hfin
                    # after both tiles of the last layer: transposes
                    if last:
                        for t in range(TT):
                            hT_cur[t], hT8_cur[t] = transpose_cast(
                                h_cur[t], head_scale, f"fin{t}", final=True
                            )

            # ---- head: own 256 tokens x full vocab, streamed fp8 weights ----
            with ExitStack() as ctxB:
                psH = ctxB.enter_context(
                    tc.tile_pool(name="psH", bufs=6, space="PSUM")
                )

                def head_group(q, t, wq):
                    cols = min(QV, V - q * QV)
                    if DRP:
                        pd = psH.tile([128, QV], F32, tag="psH", name=f"pd{q}_{t}")
                        for j in range(DRP):
                            nc.tensor.matmul(
                                pd[:],
                                lhsT=hT8_cur[t][:, 2 * j : 2 * j + 2, :],
                                rhs=wq[:, 2 * j : 2 * j + 2, :],
                                start=(j == 0),
                                stop=(j == DRP - 1),
                                perf_mode=DR,
                            )
                    pb = psH.tile([128, QV], F32, tag="psH", name=f"pb{q}_{t}")
                    for kt in range(DRKT, KT):
                        nc.tensor.matmul(
                            pb[:],
                            lhsT=hT_cur[t][:, kt * 128 : (kt + 1) * 128],
                            rhs=wq[:, kt, :],
                            start=(kt == DRKT),
                            stop=(kt == KT - 1),
                        )
                    o_t = outp.tile([128, QV], F32, tag="ostg", name=f"o{q}_{t}")
                    if DRP:
                        o_d = outp.tile(
                            [128, QV], F32, tag="odr", name=f"od{q}_{t}"
                        )
                        nc.scalar.activation(
                            o_d[:], pd[:], AF.Copy, scale=float(head_scale)
                        )
                        nc.vector.tensor_add(o_t[:], o_d[:], pb[:])
                    else:
                        nc.scalar.copy(o_t[:], pb[:])
                    nc.sync.dma_start(
                        out[t * 128 : (t + 1) * 128, q * QV : q * QV + cols],
                        o_t[:, :cols],
                    )

                # warm-up: first chunks on tile 0 only, while tile 1's final
                # LN/transposes finish; revisit their tile-1 groups at the end
                WARM = 3
                for q in range(NQ):
                    wq = wqp.tile([128, KT, QV], WDT, tag="wq", name=f"wq{q}")
                    nc.sync.dma_start(wq[:], hw_[q])
                    for t in [0] if q < WARM else range(TT):
                        head_group(q, t, wq)
                for q in range(WARM):
                    wq = wqp.tile([128, KT, QV], WDT, tag="wq", name=f"wq{q}b")
                    nc.sync.dma_start(wq[:], hw_[q])
                    head_group(q, 1, wq)

    return nc


def _ternary(wmat):
    """Exact {-1,0,1} ternary tensor + fp32 scale, matching the reference."""
    w = np.asarray(wmat, dtype=np.float32)
    s = np.mean(np.abs(w), dtype=np.float32)
    t = np.clip(np.rint(w / (s + np.float32(1e-8))), -1.0, 1.0).astype(np.float32)
    return t, float(s)


_NC_CACHE = {}
_LAST_RESULTS = None


def kernel(**inputs):
    global _LAST_RESULTS
    cfg = CFG_FULL
    L, H, NC, TT, V, QV, NQ = (
        cfg["L"], cfg["H"], cfg["NC"], cfg["TT"], cfg["V"], cfg["QV"], cfg["NQ"],
    )
    KT = H // 128
    TPC = TT * 128  # tokens per core
    BF = ml_dtypes.bfloat16
    F8 = ml_dtypes.float8_e4m3fn
    fp8_w = not bool(int(os.environ.get("TRIKERNEL_BF16_W", "0")))
    use_dr = fp8_w and not bool(int(os.environ.get("TRIKERNEL_NO_DR", "0")))
    WNP = F8 if fp8_w else BF

    ids = np.asarray(inputs["input_ids"]).astype(np.int64).reshape(-1)
    embed = np.asarray(inputs["embed"], dtype=np.float32)
    layer_w = np.asarray(inputs["layer_w"], dtype=np.float32)
    layer_b = np.asarray(inputs["layer_b"], dtype=np.float32)
    ln_g = np.asarray(inputs["ln_g"], dtype=np.float32)
    ln_b = np.asarray(inputs["ln_b"], dtype=np.float32)
    final_g = np.asarray(inputs["final_g"], dtype=np.float32)
    final_b = np.asarray(inputs["final_b"], dtype=np.float32)
    head_w = np.asarray(inputs["head_w"], dtype=np.float32)

    # trivial-affine specialization: the LN scale/shift and layer bias are
    # identity in this model instance; skip them on-chip when so.
    triv_ln = bool(
        np.all(ln_g == 1.0) and np.all(ln_b == 0.0) and np.all(layer_b == 0.0)
        and np.all(final_g == 1.0) and np.all(final_b == 0.0)
    )

    h0_full = embed[ids]  # [NTOK, H] fp32

    scales = []
    wT = np.empty([L, KT, 128, H], dtype=WNP)
    for l in range(L):
        t, s = _ternary(layer_w[l])
        scales.append(s)
        wT[l] = np.ascontiguousarray(t.T).reshape(KT, 128, H).astype(WNP)
    th, head_scale = _ternary(head_w)
    # head weights, vocab padded to NQ*QV, laid out so each [128, KT, QV]
    # chunk is a single contiguous 8KB-per-partition DMA: hw8[q, p, kt, v]
    thT = np.zeros((H, NQ * QV), dtype=np.float32)
    thT[:, :V] = th.T
    hw8 = np.ascontiguousarray(
        thT.reshape(KT, 128, NQ, QV).transpose(2, 1, 0, 3)
    ).astype(WNP)

    key = (id(cfg), tuple(scales), head_scale, triv_ln, fp8_w, use_dr)
    if key not in _NC_CACHE:
        _NC_CACHE.clear()
        nc = build_nc(cfg, scales, head_scale, triv_ln, fp8_w, use_dr)
        # Bacc.finalize runs the TRN2 legalization passes (1-wait-per-
        # instruction event-semaphore split, matmul->ldweights wait motion,
        # register allocation). The PJRT exec path serializes nc as-is.
        nc.finalize()
        _NC_CACHE[key] = nc
    nc = _NC_CACHE[key]

    common = {
        "w": wT,
        "hw": hw8,
        "ident": np.eye(128, dtype=np.float32),
        "eps": np.full((128, 1), EPS, np.float32),
    }
    if not triv_ln:
        common.update(
            lng=ln_g.astype(BF),
            lnb=ln_b.astype(BF),
            lbias=layer_b.astype(BF),
            fing=final_g.astype(BF),
            finb=final_b.astype(BF),
        )
    in_maps = []
    for c in range(NC):
        h0c = np.ascontiguousarray(
            h0_full[c * TPC : (c + 1) * TPC].reshape(TT, 128, H)
        )
        # host-side pre-transpose of the layer-0 lhsT (scaled, bf16)
        h0T = np.ascontiguousarray(
            (h0c.reshape(TT, 128, KT, 128).transpose(0, 3, 2, 1)
             * np.float32(scales[0])).reshape(TT, 128, H)
        ).astype(BF)
        in_maps.append(dict(common, h0=h0c, h0T=h0T))

    trace = bool(int(os.environ.get("TRIKERNEL_TRACE", "0")))
    res = run_bass_kernel_spmd(nc, in_maps, core_ids=list(range(NC)), trace=trace)
    _LAST_RESULTS = res

    full = np.concatenate(
        [np.asarray(res.results[c]["out"]) for c in range(NC)], axis=0
    )  # [NTOK, V]
    return full.reshape(2, 1024, 32000).astype(np.float32)


# revision 8
# speedup vs baseline: 1.0286x; 1.0286x over previous
"""Trainium2 Bass kernel: 8-layer ternary (BitNet-1.58) dense transformer.

Model (per reference):
    h = embed[input_ids]                                  # (B=2, S=1024, H=2048)
    8x: y = h @ ternary(W_l)^T + b_l ; h = LN(y + h)*g+b  # H=2048
    h = LN(h)*final_g + final_b
    logits = h @ ternary(head_W)^T                        # (B, S, V=32000)

Sharding over 8 NeuronCores (fully local, no collectives):
  - Layers: data-parallel over the 2048 tokens (256 tokens/core). Each core
    streams the full (bf16 ternary) layer weights.
  - Head: ALSO data-parallel over tokens: each core computes its own 256
    tokens x the full 32000-entry vocab. The ternary head weights are sent
    as exact {-1,0,+1} fp8(e4m3) and streamed chunk-by-chunk, overlapped
    with compute. No AllGather.

Head matmul runs mixed-precision: k-tiles 0..5 via fp8 DoubleRow (2 k-tiles
per instruction, activations rounded to e4m3), k-tiles 6..15 as bf16
activations x fp8 weights at full precision. The e4m3 rounding of 6/16 of
the contraction costs ~1.6e-2 relative error on the logits (vs the 2e-2
budget) and saves ~16% of the dominant matmul stream time.
"""

import os
import sys

import numpy as np

try:
    import concourse.bass as bass
except ImportError:  # grading container should have it on sys.path already
    sys.path.insert(0, "/opt/trn_rl_repo")
    import concourse.bass as bass

import ml_dtypes
import concourse.mybir as mybir
import concourse.tile as tile
from concourse import bacc
from concourse.bass_utils import run_bass_kernel_spmd
from contextlib import ExitStack

F32 = mybir.dt.float32
BF16 = mybir.dt.bfloat16
FP8E4 = mybir.dt.float8e4
AX = mybir.AxisListType
OP = mybir.AluOpType
AF = mybir.ActivationFunctionType
DR = mybir.MatmulPerfMode.DoubleRow
EPS = 1e-5

# Full-size problem config (B=2, S=1024 -> 2048 tokens, 256/core).
# Head: vocab padded 32000 -> 63*512; k-tiles 0..DRKT-1 run as fp8 DoubleRow.
CFG_FULL = dict(L=8, H=2048, NC=8, TT=2, V=32000, QV=512, NQ=63, CH=512, DRKT=6)


def build_nc(cfg, scales, head_scale, triv_ln, fp8_w, use_dr,
             plain_tail=False, no_ttr=False):
    L, H, NC, TT = cfg["L"], cfg["H"], cfg["NC"], cfg["TT"]
    V, QV, NQ, CH, DRKT = cfg["V"], cfg["QV"], cfg["NQ"], cfg["CH"], cfg["DRKT"]
    KT = H // 128
    KQ = KT // 4  # k-tiles per layer-weight quarter
    NCH = H // CH
    DRP = DRKT // 2
    if not use_dr:
        DRKT = DRP = 0
    assert H % CH == 0 and NQ * QV >= V
    WDT = FP8E4 if fp8_w else BF16

    nc = bacc.Bacc("TRN2", target_bir_lowering=False, debug=False, num_devices=NC)
    h0 = nc.declare_dram_parameter("h0", [TT, 128, H], F32, isOutput=False)
    h0T = nc.declare_dram_parameter("h0T", [TT, 128, H], BF16, isOutput=False)
    w_ = nc.declare_dram_parameter("w", [L, KT, 128, H], WDT, isOutput=False)
    if not triv_ln:
        lng = nc.declare_dram_parameter("lng", [L, H], BF16, isOutput=False)
        lnb = nc.declare_dram_parameter("lnb", [L, H], BF16, isOutput=False)
        lbias = nc.declare_dram_parameter("lbias", [L, H], BF16, isOutput=False)
        fing = nc.declare_dram_parameter("fing", [H], BF16, isOutput=False)
        finb = nc.declare_dram_parameter("finb", [H], BF16, isOutput=False)
    hw_ = nc.declare_dram_parameter("hw", [NQ, 128, KT, QV], WDT, isOutput=False)
    ident_d = nc.declare_dram_parameter("ident", [128, 128], F32, isOutput=False)
    eps_d = nc.declare_dram_parameter("eps", [128, 1], F32, isOutput=False)
    out = nc.declare_dram_parameter("out", [TT * 128, V], F32, isOutput=True)

    with tile.TileContext(nc) as tc:
        with ExitStack() as ctx0:
            consts = ctx0.enter_context(tc.tile_pool(name="consts", bufs=1))
            state = ctx0.enter_context(tc.tile_pool(name="state", bufs=4))
            hTp = ctx0.enter_context(tc.tile_pool(name="hT", bufs=2))
            hT8p = ctx0.enter_context(tc.tile_pool(name="hT8", bufs=2))
            wqp = ctx0.enter_context(tc.tile_pool(name="wq", bufs=3))
            outp = ctx0.enter_context(tc.tile_pool(name="outstg", bufs=4))
            smp = ctx0.enter_context(tc.tile_pool(name="small", bufs=16))

            ident = consts.tile([128, 128], F32)
            nc.sync.dma_start(ident[:], ident_d[:])
            eps_t = consts.tile([128, 1], F32)
            nc.sync.dma_start(eps_t[:], eps_d[:])

            h_cur = []
            hT_cur = []
            for t in range(TT):
                # pre-transposed & pre-scaled (layer-0 ternary scale) lhsT
                hTt = hTp.tile([128, H], BF16, tag="hT", name=f"hT_p{t}")
                nc.sync.dma_start(hTt[:], h0T[t])
                hT_cur.append(hTt)
                st = state.tile([128, H], F32, name=f"hinit{t}", tag="state")
                nc.sync.dma_start(st[:], h0[t])
                h_cur.append(st)
            hT8_cur = [None] * TT

            with ExitStack() as ctxA:
                zpool = ctxA.enter_context(tc.tile_pool(name="z", bufs=2))
                wp = ctxA.enter_context(tc.tile_pool(name="w", bufs=6))
                sqp = ctxA.enter_context(tc.tile_pool(name="sq", bufs=2))
                gbp = None
                if not triv_ln:
                    gbp = ctxA.enter_context(tc.tile_pool(name="gb", bufs=2))
                psT = ctxA.enter_context(
                    tc.tile_pool(name="psT", bufs=2, space="PSUM")
                )
                psY = ctxA.enter_context(
                    tc.tile_pool(name="psY", bufs=NCH, space="PSUM")
                )

                def transpose_cast(src_f32, scale_imm, name, final=False):
                    """h [128tok, H] f32 -> hT [128feat-in-blk, (kt,128tok)]
                    bf16 * s, split into 2 halves so the scalar copy of half
                    0 overlaps the PE transposes of half 1. For the final
                    (head) activations also emit an UNSCALED e4m3 copy of
                    k-tiles 0..DRKT-1 for the DoubleRow part."""
                    dst = hTp.tile([128, H], BF16, tag="hT", name=f"hT{name}")
                    dst8 = None
                    if final and DRKT:
                        dst8 = hT8p.tile(
                            [128, DRKT, 128], FP8E4, tag="hT8", name=f"hT8{name}"
                        )
                    for hf in range(2):
                        pT = psT.tile(
                            [128, H // 2], F32, tag="psT", name=f"pT{name}_{hf}"
                        )
                        for k in range(KT // 2):
                            kt = hf * (KT // 2) + k
                            nc.tensor.transpose(
                                pT[:, k * 128 : (k + 1) * 128],
                                src_f32[:, kt * 128 : (kt + 1) * 128],
                                ident[:],
                            )
                        lo = hf * (H // 2)
                        if dst8 is not None and hf == 0:
                            nc.scalar.activation(
                                dst8[:].rearrange("p a b -> p (a b)"),
                                pT[:, : DRKT * 128],
                                AF.Copy,
                            )
                            nc.scalar.activation(
                                dst[:, DRKT * 128 : H // 2],
                                pT[:, DRKT * 128 :],
                                AF.Copy,
                                scale=float(scale_imm),
                            )
                        else:
                            nc.scalar.activation(
                                dst[:, lo : lo + H // 2],
                                pT[:],
                                AF.Copy,
                                scale=float(scale_imm),
                            )
                    return dst, dst8

                def ln_finish(z, S_ap, SS_ap, g_t, b_t, name):
                    S = smp.tile([128, 1], F32, tag="s0", name=f"S{name}")
                    SS = smp.tile([128, 1], F32, tag="s1", name=f"SS{name}")
                    nc.vector.tensor_reduce(S[:], S_ap, axis=AX.X, op=OP.add)
                    nc.vector.tensor_reduce(SS[:], SS_ap, axis=AX.X, op=OP.add)
                    negmean = smp.tile([128, 1], F32, tag="s2", name=f"nm{name}")
                    nc.vector.tensor_scalar_mul(negmean[:], S[:], -1.0 / H)
                    msq = smp.tile([128, 1], F32, tag="s3", name=f"msq{name}")
                    nc.vector.tensor_scalar_mul(msq[:], SS[:], 1.0 / H)
                    var = smp.tile([128, 1], F32, tag="s4", name=f"var{name}")
                    nc.vector.tensor_tensor(var[:], negmean[:], negmean[:], OP.mult)
                    nc.vector.tensor_tensor(var[:], msq[:], var[:], OP.subtract)
                    std = smp.tile([128, 1], F32, tag="s5", name=f"std{name}")
                    nc.scalar.activation(std[:], var[:], AF.Sqrt, bias=eps_t[:])
                    rstd = smp.tile([128, 1], F32, tag="s6", name=f"rstd{name}")
                    nc.vector.reciprocal(rstd[:], std[:])
                    hn = state.tile([128, H], F32, tag="state", name=f"h{name}")
                    nc.vector.tensor_scalar(
                        hn[:], z[:], negmean[:], rstd[:], OP.add, OP.mult
                    )
                    if g_t is not None:
                        nc.vector.tensor_tensor(hn[:], hn[:], g_t[:], OP.mult)
                        nc.vector.tensor_tensor(hn[:], hn[:], b_t[:], OP.add)
                    return hn

                def stats_of(src, name):
                    """LN stats for an existing tensor (final LN input)."""
                    sums = smp.tile(
                        [128, 2 * NCH], F32, tag="sums", name=f"sm{name}"
                    )
                    nc.vector.tensor_reduce(
                        sums[:, 0:1], src[:], axis=AX.X, op=OP.add
                    )
                    for i in range(NCH):
                        sq = sqp.tile(
                            [128, CH], BF16, tag="sq", name=f"sq{name}_{i}"
                        )
                        nc.scalar.activation(
                            sq[:],
                            src[:, i * CH : (i + 1) * CH],
                            AF.Square,
                            accum_out=sums[:, NCH + i : NCH + i + 1],
                        )
                    return sums

                fg = fb = None
                if not triv_ln:
                    fg = gbp.tile([128, H], BF16, tag="g", name="gfin")
                    nc.sync.dma_start(fg[:], fing[None, :].to_broadcast((128, H)))
                    fb = gbp.tile([128, H], BF16, tag="b", name="bfin")
                    nc.sync.dma_start(fb[:], finb[None, :].to_broadcast((128, H)))

                for l in range(L):
                    w_q = []
                    for hf in range(4):
                        wt = wp.tile([128, KQ, H], WDT, tag="w", name=f"w{l}_{hf}")
                        nc.sync.dma_start(
                            wt[:],
                            w_[l, hf * KQ : (hf + 1) * KQ].rearrange(
                                "k p o -> p k o"
                            ),
                        )
                        w_q.append(wt)
                    g_t = b_t = bias_t = None
                    if not triv_ln:
                        g_t = gbp.tile([128, H], BF16, tag="g", name=f"g{l}")
                        nc.sync.dma_start(
                            g_t[:], lng[l][None, :].to_broadcast((128, H))
                        )
                        b_t = gbp.tile([128, H], BF16, tag="b", name=f"b{l}")
                        nc.sync.dma_start(
                            b_t[:], lnb[l][None, :].to_broadcast((128, H))
                        )
                        bias_t = gbp.tile(
                            [128, H], BF16, tag="bias", name=f"bias{l}"
                        )
                        nc.sync.dma_start(
                            bias_t[:], lbias[l][None, :].to_broadcast((128, H))
                        )

                    last = (l == L - 1) and not plain_tail
                    next_scale = scales[l + 1] if l + 1 < L else None
                    for t in range(TT):
                        hTt = hT_cur[t]
                        ps = [
                            psY.tile([128, CH], F32, tag="psY", name=f"ps{l}_{t}_{i}")
                            for i in range(NCH)
                        ]
                        for half in range(2):
                            for kt in range(KT):
                                wt = w_q[kt // KQ]
                                for i in (2 * half, 2 * half + 1):
                                    nc.tensor.matmul(
                                        ps[i][:],
                                        lhsT=hTt[:, kt * 128 : (kt + 1) * 128],
                                        rhs=wt[:, kt % KQ, i * CH : (i + 1) * CH],
                                        start=(kt == 0),
                                        stop=(kt == KT - 1),
                                    )
                        z = zpool.tile([128, H], F32, tag="z", name=f"z{l}_{t}")
                        sums = smp.tile(
                            [128, 2 * NCH], F32, tag="sums", name=f"sm{l}_{t}"
                        )
                        resid = h_cur[t]
                        if not triv_ln:
                            hb = zpool.tile(
                                [128, H], F32, tag="hb", name=f"hb{l}_{t}"
                            )
                            nc.vector.tensor_tensor(
                                hb[:], h_cur[t][:], bias_t[:], OP.add
                            )
                            resid = hb
                        for i in range(NCH):
                            if no_ttr:
                                nc.vector.tensor_add(
                                    z[:, i * CH : (i + 1) * CH],
                                    ps[i][:],
                                    resid[:, i * CH : (i + 1) * CH],
                                )
                            else:
                                # fused: z = ps + resid AND chunk-sum accum
                                nc.vector.tensor_tensor_reduce(
                                    z[:, i * CH : (i + 1) * CH],
                                    ps[i][:],
                                    resid[:, i * CH : (i + 1) * CH],
                                    1.0,
                                    0.0,
                                    OP.add,
                                    OP.add,
                                    accum_out=sums[:, i : i + 1],
                                )
                        if no_ttr:
                            nc.vector.tensor_reduce(
                                sums[:, 0:1], z[:], axis=AX.X, op=OP.add
                            )
                        for i in range(NCH):
                            sq = sqp.tile(
                                [128, CH], BF16, tag="sq", name=f"sq{l}_{t}_{i}"
                            )
                            nc.scalar.activation(
                                sq[:],
                                z[:, i * CH : (i + 1) * CH],
                                AF.Square,
                                accum_out=sums[:, NCH + i : NCH + i + 1],
                            )
                        hn = ln_finish(
                            z,
                            sums[:, 0:1] if no_ttr else sums[:, 0:NCH],
                            sums[:, NCH : 2 * NCH],
                            g_t, b_t, f"{l}_{t}",
                        )
                        h_cur[t] = hn
                        if l < L - 1:
                            hT_cur[t], _ = transpose_cast(
                                hn, next_scale, f"{l}_{t}"
                            )
                        elif last:
                            # final LN chain emitted immediately (vector/
                            # scalar work overlaps the other tile's matmuls)
                            fsums = stats_of(hn, f"f{t}")
                            hfin = ln_finish(
                                hn, fsums[:, 0:1], fsums[:, NCH : 2 * NCH],
                                fg, fb, f"fin{t}",
                            )
                            h_cur[t] = hfin
                    # after both tiles of the last layer: transposes
                    if last:
                        for t in range(TT):
                            hT_cur[t], hT8_cur[t] = transpose_cast(
                                h_cur[t], head_scale, f"fin{t}", final=True
                            )
                if plain_tail:
                    for t in range(TT):
                        hn = h_cur[t]
                        fsums = stats_of(hn, f"f{t}")
                        hfin = ln_finish(
                            hn, fsums[:, 0:1], fsums[:, NCH : 2 * NCH],
                            fg, fb, f"fin{t}",
                        )
                        h_cur[t] = hfin
                        hT_cur[t], hT8_cur[t] = transpose_cast(
                            hfin, head_scale, f"fin{t}", final=True
                        )

            # ---- head: own 256 tokens x full vocab, streamed fp8 weights ----
            with ExitStack() as ctxB:
                psH = ctxB.enter_context(
                    tc.tile_pool(name="psH", bufs=6, space="PSUM")
                )

                def head_group(q, t, wq):
                    cols = min(QV, V - q * QV)
                    if DRP:
                        pd = psH.tile([128, QV], F32, tag="psH", name=f"pd{q}_{t}")
                        for j in range(DRP):
                            nc.tensor.matmul(
                                pd[:],
                                lhsT=hT8_cur[t][:, 2 * j : 2 * j + 2, :],
                                rhs=wq[:, 2 * j : 2 * j + 2, :],
                                start=(j == 0),
                                stop=(j == DRP - 1),
                                perf_mode=DR,
                            )
                    pb = psH.tile([128, QV], F32, tag="psH", name=f"pb{q}_{t}")
                    for kt in range(DRKT, KT):
                        nc.tensor.matmul(
                            pb[:],
                            lhsT=hT_cur[t][:, kt * 128 : (kt + 1) * 128],
                            rhs=wq[:, kt, :],
                            start=(kt == DRKT),
                            stop=(kt == KT - 1),
                        )
                    o_t = outp.tile([128, QV], F32, tag="ostg", name=f"o{q}_{t}")
                    if DRP:
                        o_d = outp.tile(
                            [128, QV], F32, tag="odr", name=f"od{q}_{t}"
                        )
                        nc.scalar.activation(
                            o_d[:], pd[:], AF.Copy, scale=float(head_scale)
                        )
                        nc.vector.tensor_add(o_t[:], o_d[:], pb[:])
                    else:
                        nc.scalar.copy(o_t[:], pb[:])
                    nc.sync.dma_start(
                        out[t * 128 : (t + 1) * 128, q * QV : q * QV + cols],
                        o_t[:, :cols],
                    )

                # warm-up: first chunks on tile 0 only, while tile 1's final
                # LN/transposes finish; revisit their tile-1 groups at the end
                WARM = 0 if plain_tail else 3
                for q in range(NQ):
                    wq = wqp.tile([128, KT, QV], WDT, tag="wq", name=f"wq{q}")
                    nc.sync.dma_start(wq[:], hw_[q])
                    for t in [0] if q < WARM else range(TT):
                        head_group(q, t, wq)
                for q in range(WARM):
                    wq = wqp.tile([128, KT, QV], WDT, tag="wq", name=f"wq{q}b")
                    nc.sync.dma_start(wq[:], hw_[q])
                    head_group(q, 1, wq)

    return nc


def _ternary(wmat):
    """Exact {-1,0,1} ternary tensor + fp32 scale, matching the reference."""
    w = np.asarray(wmat, dtype=np.float32)
    s = np.mean(np.abs(w), dtype=np.float32)
    t = np.clip(np.rint(w / (s + np.float32(1e-8))), -1.0, 1.0).astype(np.float32)
    return t, float(s)


_NC_CACHE = {}
_LAST_RESULTS = None


def kernel(**inputs):
    global _LAST_RESULTS
    cfg = CFG_FULL
    L, H, NC, TT, V, QV, NQ = (
        cfg["L"], cfg["H"], cfg["NC"], cfg["TT"], cfg["V"], cfg["QV"], cfg["NQ"],
    )
    KT = H // 128
    TPC = TT * 128  # tokens per core
    BF = ml_dtypes.bfloat16
    F8 = ml_dtypes.float8_e4m3fn
    fp8_w = not bool(int(os.environ.get("TRIKERNEL_BF16_W", "0")))
    use_dr = fp8_w and not bool(int(os.environ.get("TRIKERNEL_NO_DR", "0")))
    plain_tail = bool(int(os.environ.get("TRIKERNEL_PLAIN_TAIL", "0")))
    no_ttr = bool(int(os.environ.get("TRIKERNEL_NO_TTR", "0")))
    if bool(int(os.environ.get("TRIKERNEL_QV500", "0"))):
        cfg = dict(cfg, QV=500, NQ=64, DRKT=0)
        L, H, NC, TT, V, QV, NQ = (
            cfg["L"], cfg["H"], cfg["NC"], cfg["TT"], cfg["V"], cfg["QV"],
            cfg["NQ"],
        )
    WNP = F8 if fp8_w else BF

    ids = np.asarray(inputs["input_ids"]).astype(np.int64).reshape(-1)
    embed = np.asarray(inputs["embed"], dtype=np.float32)
    layer_w = np.asarray(inputs["layer_w"], dtype=np.float32)
    layer_b = np.asarray(inputs["layer_b"], dtype=np.float32)
    ln_g = np.asarray(inputs["ln_g"], dtype=np.float32)
    ln_b = np.asarray(inputs["ln_b"], dtype=np.float32)
    final_g = np.asarray(inputs["final_g"], dtype=np.float32)
    final_b = np.asarray(inputs["final_b"], dtype=np.float32)
    head_w = np.asarray(inputs["head_w"], dtype=np.float32)

    # trivial-affine specialization: the LN scale/shift and layer bias are
    # identity in this model instance; skip them on-chip when so.
    triv_ln = bool(
        np.all(ln_g == 1.0) and np.all(ln_b == 0.0) and np.all(layer_b == 0.0)
        and np.all(final_g == 1.0) and np.all(final_b == 0.0)
    )

    h0_full = embed[ids]  # [NTOK, H] fp32

    scales = []
    wT = np.empty([L, KT, 128, H], dtype=WNP)
    for l in range(L):
        t, s = _ternary(layer_w[l])
        scales.append(s)
        wT[l] = np.ascontiguousarray(t.T).reshape(KT, 128, H).astype(WNP)
    th, head_scale = _ternary(head_w)
    # head weights, vocab padded to NQ*QV, laid out so each [128, KT, QV]
    # chunk is a single contiguous 8KB-per-partition DMA: hw8[q, p, kt, v]
    thT = np.zeros((H, NQ * QV), dtype=np.float32)
    thT[:, :V] = th.T
    hw8 = np.ascontiguousarray(
        thT.reshape(KT, 128, NQ, QV).transpose(2, 1, 0, 3)
    ).astype(WNP)

    key = (tuple(sorted(cfg.items())), tuple(scales), head_scale, triv_ln,
           fp8_w, use_dr, plain_tail, no_ttr)
    if key not in _NC_CACHE:
        _NC_CACHE.clear()
        nc = build_nc(cfg, scales, head_scale, triv_ln, fp8_w, use_dr,
                      plain_tail=plain_tail, no_ttr=no_ttr)
        # Bacc.finalize runs the TRN2 legalization passes (1-wait-per-
        # instruction event-semaphore split, matmul->ldweights wait motion,
        # register allocation). The PJRT exec path serializes nc as-is.
        nc.finalize()
        _NC_CACHE[key] = nc
    nc = _NC_CACHE[key]

    common = {
        "w": wT,
        "hw": hw8,
        "ident": np.eye(128, dtype=np.float32),
        "eps": np.full((128, 1), EPS, np.float32),
    }
    if not triv_ln:
        common.update(
            lng=ln_g.astype(BF),
            lnb=ln_b.astype(BF),
            lbias=layer_b.astype(BF),
            fing=final_g.astype(BF),
            finb=final_b.astype(BF),
        )
    in_maps = []
    for c in range(NC):
        h0c = np.ascontiguousarray(
            h0_full[c * TPC : (c + 1) * TPC].reshape(TT, 128, H)
        )
        # host-side pre-transpose of the layer-0 lhsT (scaled, bf16)
        h0T = np.ascontiguousarray(
            (h0c.reshape(TT, 128, KT, 128).transpose(0, 3, 2, 1)
             * np.float32(scales[0])).reshape(TT, 128, H)
        ).astype(BF)
        in_maps.append(dict(common, h0=h0c, h0T=h0T))

    trace = bool(int(os.environ.get("TRIKERNEL_TRACE", "0")))
    res = run_bass_kernel_spmd(nc, in_maps, core_ids=list(range(NC)), trace=trace)
    _LAST_RESULTS = res

    full = np.concatenate(
        [np.asarray(res.results[c]["out"]) for c in range(NC)], axis=0
    )  # [NTOK, V]
    return full.reshape(2, 1024, 32000).astype(np.float32)
